# revision 53
# baseline (speedup 1.0000x reference)
"""Trainium2 Bass kernel for AttentionConstrainedLoss.

Contract: kernel(atten_map [16,1600,2048] f32, gt_bboxes [16,64,7] f32) -> scalar f32.

Strategy (data-parallel over batch, 2 scenes per core on 8 cores):
  - per cell: variance over the 2048 feature dim streamed in 26 chunks of
    <=128 rows. Chunks split ACT copy+square accum (12) / DVE bn_stats (14)
    so BOTH engines stay under the ~62us DMA floor (26 MB @ ~420 GB/s).
  - box->grid assignment flag[g] = odd(#covering boxes) ? last covering box
    : -1. The inside test and the closed-form nearest-cell rule (snap the
    center to the grid, then dist^2-1 <= 0) are linear in the 6-dim grid
    basis [x^2, y^2, xy, x, y, 1]: PE matmuls produce (S, T, near) columns
    per box, cells on partitions.
  - engine split honors hardware limits: only DVE reads PSUM and compares
    (one wide is_le per round, is_equal onehots, max-reduce for the last-
    box id); the GpSimd/Pool engine - whose library has ONLY tensor_tensor
    mult/add/sub (plus iota and SWDGE cast-DMAs) - does all {0,1} boolean
    algebra, count trees, and the parity/flag chain, exactly in bf16
    (every value <= 128 is bf16-exact; the round-to-nearest magic is 128).
  - per-chunk stats land in a bf16 rhs (batched gpsimd cast-DMA for the
    early ACT chunks, tiny DVE converts for the late ones); onehots are
    bf16 so the segment-sum matmuls are single-pass bf16 (PSUM still
    accumulates f32). Both scenes accumulate into ONE [128,4] PSUM
    (var_bn, sumsq, K2*sum^2, count); host finishes in f64.
"""

from contextlib import ExitStack

import numpy as np

_CACHE = {}

# problem constants (hardcoded per spec)
B, G, D, M = 16, 1600, 2048, 64
NCORES = 8
BPC = B // NCORES          # batches per core = 2
ROWS = BPC * G             # 3200 rows of [D] per core
NCH = 13                   # 13 chunks of <=128 cells per scene (12*128 + 64)
NCHUNK = BPC * NCH         # 26 x-chunks per core

F2 = float(np.float64(102.4) / np.float64(40.0))   # 2.56 cell pitch

# chunks on the ACT copy+square path; rest use DVE bn_stats. Strict
# alternation at the stream tail (19..23) plus DVE on the split-DMA'd
# 24/25 keeps the final chunks' compute off the critical path.
ACT_CHUNKS = frozenset((0, 2, 4, 6, 8, 10, 12, 14, 16, 19, 21, 23))
LATE_ACT = 17              # ACT chunks > LATE_ACT convert stats on DVE


def _build_program(xbufs=12, ntail=8, dvecomb=2, ohsplit=7):
    import concourse.bacc as bacc
    import concourse.tile as tile
    from concourse import mybir

    f32 = mybir.dt.float32
    bf16 = mybir.dt.bfloat16
    op = mybir.AluOpType
    AF = mybir.ActivationFunctionType
    X = mybir.AxisListType

    nc = bacc.Bacc("TRN2", target_bir_lowering=False, debug=False,
                   enable_asserts=False, num_devices=NCORES)

    x_d = nc.declare_dram_parameter("x", [ROWS, D], f32, isOutput=False)
    bb_d = nc.declare_dram_parameter("bb", [2 * M, 7], f32, isOutput=False)
    out_d = nc.declare_dram_parameter("out", [128, 4], f32, isOutput=True)

    with tile.TileContext(nc) as tc, ExitStack() as ctx:
        singles = ctx.enter_context(tc.tile_pool(name="singles", bufs=1))
        xpool = ctx.enter_context(tc.tile_pool(name="x", bufs=1))
        bnpool = ctx.enter_context(tc.tile_pool(name="bn", bufs=4))
        mskpool = ctx.enter_context(tc.tile_pool(name="msk", bufs=3))
        tpps = ctx.enter_context(tc.tile_pool(name="tpps", bufs=2,
                                              space="PSUM"))
        mmps = ctx.enter_context(tc.tile_pool(name="mmps", bufs=1, space="PSUM"))
        segps = ctx.enter_context(tc.tile_pool(name="segps", bufs=1,
                                               space="PSUM"))

        # ---------------- x stream: first xbufs triggers up front ------------
        xap = x_d.ap()
        xts = {}

        def emit_dma(c):
            b, t = c // NCH, c % NCH
            csz = 128 if t < NCH - 1 else G - 128 * (NCH - 1)
            r0 = b * G + t * 128
            if c >= NCHUNK - ntail:
                # dedicated tail buffers: no ring reuse, so the last
                # triggers carry no consumer-WAR waits and fire at DMA pace
                xt = singles.tile([128, D], f32, tag=f"xtl{c}", name=f"xtl{c}")
            else:
                xt = xpool.tile([128, D], f32, tag="xt", name="xt", bufs=xbufs)
            nc.sync.dma_start(out=xt[:csz, :], in_=xap[r0:r0 + csz, :])
            xts[c] = xt

        # bb first on Sync: the box-coef chain (ACT sincos) needs it
        # ~3us earlier than the gpsimd SWDGE path can deliver
        bb = singles.tile([128, 7], f32)
        nc.sync.dma_start(out=bb, in_=bb_d.ap())

        for c in range(min(xbufs, NCHUNK)):
            emit_dma(c)

        # ---------------- gpsimd constants (iotas) ---------------------------
        g_t = singles.tile([128, NCH], f32)    # cell id g = 128*t + p
        nc.gpsimd.iota(g_t, pattern=[[128, NCH]], base=0,
                       channel_multiplier=1, allow_small_or_imprecise_dtypes=True)
        ident = singles.tile([128, 128], f32)
        nc.gpsimd.iota(ident, pattern=[[1, 128]], base=0, channel_multiplier=-1,
                       allow_small_or_imprecise_dtypes=True)
        w128b = singles.tile([128, 128], bf16)  # box col j -> j+1 (<=128: exact)
        nc.gpsimd.iota(w128b, pattern=[[1, 128]], base=1, channel_multiplier=0,
                       allow_small_or_imprecise_dtypes=True)
        iotaexp = singles.tile([128, NCH, 128], bf16)  # 0..127 per chunk
        nc.gpsimd.iota(iotaexp, pattern=[[0, NCH], [1, 128]], base=0,
                       channel_multiplier=0, allow_small_or_imprecise_dtypes=True)

        # ---------------- DVE setup: ident fixup, small consts ---------------
        nc.vector.tensor_scalar(out=ident, in0=ident, scalar1=0.0,
                                scalar2=None, op0=op.is_equal)
        K2 = float(np.float32(-1.0 / (2048.0 * 2048.0)))
        k2c = singles.tile([128, 1], f32)
        nc.vector.memset(k2c, K2)
        # bf16 broadcast constants for the Pool flag chain
        oneb = singles.tile([128, 1], bf16)
        nc.vector.memset(oneb, 1.0)
        halfb = singles.tile([128, 1], bf16)
        nc.vector.memset(halfb, 0.5)
        fourb = singles.tile([128, 1], bf16)
        nc.vector.memset(fourb, 4.0)
        bigb = singles.tile([128, 1], bf16)    # bf16 rne magic: ulp(128)=1
        nc.vector.memset(bigb, 128.0)
        zerob = singles.tile([128, 1], bf16)
        nc.vector.memset(zerob, 0.0)

        # grid basis per cell: [x^2, y^2, xy, x, y, 1]
        r40 = float(np.float32(1.0) / np.float32(40.0))
        basis = singles.tile([128, NCH, 6], f32)
        h_t = singles.tile([128, NCH], f32)
        nc.vector.tensor_scalar(out=h_t, in0=g_t, scalar1=r40, scalar2=None,
                                op0=op.mult)
        r_t = singles.tile([128, NCH], f32)
        nc.vector.tensor_scalar(out=r_t, in0=h_t, scalar1=8388608.0,
                                scalar2=8388608.0, op0=op.add, op1=op.subtract)
        gt_t = singles.tile([128, NCH], f32)
        nc.vector.tensor_tensor(out=gt_t, in0=r_t, in1=h_t, op=op.is_gt)
        fl_t = singles.tile([128, NCH], f32)   # row index = floor(g/40)
        nc.vector.tensor_tensor(out=fl_t, in0=r_t, in1=gt_t, op=op.subtract)
        col_t = singles.tile([128, NCH], f32)  # col index = g - 40*row
        nc.vector.tensor_scalar(out=col_t, in0=fl_t, scalar1=-40.0,
                                scalar2=None, op0=op.mult)
        nc.vector.tensor_tensor(out=col_t, in0=col_t, in1=g_t, op=op.add)
        bx = basis[:, :, 3]
        by = basis[:, :, 4]
        for src, dst in ((col_t, bx), (fl_t, by)):
            nc.vector.tensor_scalar(out=dst, in0=src, scalar1=0.5,
                                    scalar2=r40, op0=op.add, op1=op.mult)
            nc.vector.tensor_scalar(out=dst, in0=dst,
                                    scalar1=float(np.float32(102.4)),
                                    scalar2=float(np.float32(-51.2)),
                                    op0=op.mult, op1=op.add)
        nc.vector.tensor_tensor(out=basis[:, :, 0], in0=bx, in1=bx, op=op.mult)
        nc.vector.tensor_tensor(out=basis[:, :, 1], in0=by, in1=by, op=op.mult)
        nc.vector.tensor_tensor(out=basis[:, :, 2], in0=bx, in1=by, op=op.mult)
        nc.vector.memset(basis[:, :, 5], 1.0)

        # per-chunk bf16 rhs: cols (var_bn, _, sumsq, K2*sum^2, 1); matmul
        # uses cols 1:5. col0 takes bn_aggr's mean (unused).
        vrhsb = singles.tile([128, NCHUNK, 5], bf16)
        nc.vector.memset(vrhsb, 0.0)
        nc.vector.memset(vrhsb[:, :, 4], 1.0)
        stats = singles.tile([128, NCHUNK, 1], f32)
        # ACT-chunk staging: (sumsq, K2*sum^2) in f32; a batched gpsimd
        # cast-DMA converts to the bf16 rhs (Pool ops cannot cast to bf16
        # cheaply per chunk). memset so dead rows cast zeros, not NaNs.
        vstg = singles.tile([128, NCHUNK, 2], f32)
        nc.vector.memset(vstg, 0.0)

        # ---------------- per-box coefs (boxes of both scenes on partitions) --
        cx, cy = bb[:, 0:1], bb[:, 1:2]
        yaw = bb[:, 6:7]

        # paired (T,S)-role coef chain: A=(sl,cw), B=(-cl,sw), midP=(midT,midS)
        ratP = singles.tile([128, 2], f32)
        nc.vector.reciprocal(ratP, bb[:, 3:5])
        nc.vector.tensor_scalar(out=ratP, in0=ratP, scalar1=F2, scalar2=1.0,
                                op0=op.mult, op1=op.max)
        nc.vector.tensor_scalar(out=ratP, in0=ratP, scalar1=6.0, scalar2=None,
                                op0=op.min)
        eeP = singles.tile([128, 2], f32)        # (el, ew)
        nc.vector.tensor_tensor(out=eeP, in0=bb[:, 3:5], in1=ratP, op=op.mult)
        sgn = singles.tile([128, 2], f32)
        nc.vector.memset(sgn[:, 0:1], -1.0)
        nc.vector.memset(sgn[:, 1:2], 1.0)
        eeN = singles.tile([128, 2], f32)        # (-el, ew)
        nc.vector.tensor_tensor(out=eeN, in0=eeP, in1=sgn, op=op.mult)

        cs = singles.tile([128, 2], f32)         # (sin, cos)
        sc = singles.tile([128, 2], f32)         # (cos, sin)
        halfpi = singles.tile([128, 1], f32)
        nc.vector.memset(halfpi, float(np.pi / 2))
        absyaw = singles.tile([128, 1], f32)
        nc.scalar.activation(absyaw, yaw, AF.Abs)
        nc.scalar.activation(cs[:, 0:1], yaw, AF.Sin)
        nc.scalar.activation(sc[:, 1:2], yaw, AF.Sin)
        # cos(x) = sin(pi/2 - |x|) keeps the Sin arg in [-pi, pi]
        nc.scalar.activation(cs[:, 1:2], absyaw, AF.Sin, bias=halfpi[:, 0:1],
                             scale=-1.0)
        nc.scalar.activation(sc[:, 0:1], absyaw, AF.Sin, bias=halfpi[:, 0:1],
                             scale=-1.0)

        A_p = singles.tile([128, 2], f32)        # (sl, cw)
        nc.vector.tensor_tensor(out=A_p, in0=cs, in1=eeP, op=op.mult)
        B_p = singles.tile([128, 2], f32)        # (-cl, sw)
        nc.vector.tensor_tensor(out=B_p, in0=sc, in1=eeN, op=op.mult)
        t1 = singles.tile([128, 2], f32)
        nc.vector.tensor_scalar(out=t1, in0=A_p, scalar1=cx, scalar2=None,
                                op0=op.mult)
        midP = singles.tile([128, 2], f32)       # (midT, midS)
        nc.vector.scalar_tensor_tensor(out=midP, in0=B_p, scalar=cy,
                                       in1=t1, op0=op.mult, op1=op.add)
        half = singles.tile([128, 1], f32)
        nc.vector.tensor_tensor(out=half, in0=eeP[:, 0:1], in1=eeP[:, 1:2],
                                op=op.mult)
        nc.vector.tensor_scalar(out=half, in0=half, scalar1=0.5, scalar2=None,
                                op0=op.mult)
        hh2 = singles.tile([128, 1], f32)
        nc.vector.tensor_tensor(out=hh2, in0=half, in1=half, op=op.mult)

        # coefP[:, role, k] on basis [x^2,y^2,xy,x,y,1]; role order (T,S)
        coefP = singles.tile([128, 2, 6], f32)
        nc.vector.tensor_tensor(out=coefP[:, :, 0], in0=A_p, in1=A_p,
                                op=op.mult)
        nc.vector.tensor_tensor(out=coefP[:, :, 1], in0=B_p, in1=B_p,
                                op=op.mult)
        nc.vector.scalar_tensor_tensor(out=coefP[:, :, 2], in0=A_p, scalar=2.0,
                                       in1=B_p, op0=op.mult, op1=op.mult)
        nc.vector.scalar_tensor_tensor(out=coefP[:, :, 3], in0=A_p, scalar=-2.0,
                                       in1=midP, op0=op.mult, op1=op.mult)
        nc.vector.scalar_tensor_tensor(out=coefP[:, :, 4], in0=B_p, scalar=-2.0,
                                       in1=midP, op0=op.mult, op1=op.mult)
        nc.vector.tensor_tensor(out=coefP[:, :, 5], in0=midP, in1=midP,
                                op=op.mult)
        nc.vector.tensor_scalar(out=coefP[:, :, 5], in0=coefP[:, :, 5],
                                scalar1=hh2[:, 0:1], scalar2=None,
                                op0=op.subtract)

        # nearest-cell coefs: snap center to grid (closed form), then
        # near(g) <=> dist^2((x,y),(nx,ny)) - 1 <= 0 as a quadratic column.
        rp = float(np.float32(1.0) / np.float32(F2))
        un = singles.tile([128, 2], f32)
        nc.vector.tensor_scalar(out=un, in0=bb[:, 0:2],
                                scalar1=float(np.float32(51.2)),
                                scalar2=rp, op0=op.add, op1=op.mult)
        rn = singles.tile([128, 2], f32)
        nc.vector.tensor_scalar(out=rn, in0=un, scalar1=8388608.0,
                                scalar2=8388608.0, op0=op.add, op1=op.subtract)
        gn = singles.tile([128, 2], f32)
        nc.vector.tensor_tensor(out=gn, in0=rn, in1=un, op=op.is_gt)
        fln = singles.tile([128, 2], f32)       # (col, row) floors
        nc.vector.tensor_tensor(out=fln, in0=rn, in1=gn, op=op.subtract)
        nxy = singles.tile([128, 2], f32)       # snapped center (nx, ny)
        nc.vector.tensor_scalar(out=nxy, in0=fln, scalar1=0.5, scalar2=F2,
                                op0=op.add, op1=op.mult)
        nc.vector.tensor_scalar(out=nxy, in0=nxy,
                                scalar1=float(np.float32(-51.2)),
                                scalar2=None, op0=op.add)
        coefN = singles.tile([128, 6], f32)
        nc.vector.memset(coefN[:, 0:2], 1.0)
        nc.vector.memset(coefN[:, 2:3], 0.0)
        nc.vector.tensor_scalar(out=coefN[:, 3:5], in0=nxy, scalar1=-2.0,
                                scalar2=None, op0=op.mult)
        nn2 = singles.tile([128, 2], f32)
        nc.vector.tensor_tensor(out=nn2, in0=nxy, in1=nxy, op=op.mult)
        nc.vector.tensor_tensor(out=coefN[:, 5:6], in0=nn2[:, 0:1],
                                in1=nn2[:, 1:2], op=op.add)
        nc.vector.tensor_scalar(out=coefN[:, 5:6], in0=coefN[:, 5:6],
                                scalar1=-1.0, scalar2=None, op0=op.add)

        # ---------------- transposes: basis chunks + coef groups --------------
        # 4 chunk-transposes share one PSUM tile so the PSUM->SBUF drain is
        # 4 ACT copies instead of 13 (the per-copy SBUF access dominates)
        basisT = singles.tile([6, NCH * 128], f32)   # [6, 1664] cells free
        for grp in range(4):
            n = min(4, NCH - 4 * grp)
            ps = tpps.tile([128, 512], f32, tag="tp")
            for k in range(n):
                t = 4 * grp + k
                nc.tensor.transpose(ps[:6, k * 128:(k + 1) * 128],
                                    basis[:, t, :], ident)
            nc.scalar.copy(basisT[:, 4 * grp * 128:(4 * grp + n) * 128],
                           ps[:6, 0:n * 128])

        # rhsST cols 3j+(0,1,2) = (T, S, near) per box j
        rhsST = singles.tile([6, 384], f32)
        stv = rhsST[:, :].rearrange("p (c three) -> p c three", three=3)
        for i in range(3):
            ps = tpps.tile([128, 512], f32, tag="tp")
            src_ap = coefP[:, i, :] if i < 2 else coefN
            nc.tensor.transpose(ps[:6, 0:128], src_ap, ident)
            nc.vector.tensor_copy(stv[:, :, i], ps[:6, 0:128])

        ohall = [singles.tile([128, NCH, 128], bf16, tag=f"oha{b}",
                              name=f"oha{b}")
                 for b in range(BPC)]

        # ---------------- mask rounds, batched 2 per PSUM round ---------------
        # PE matmul -> one wide DVE is_le (only DVE may read PSUM) ->
        # Pool {0,1} algebra: covered = S*T + near - S*T*near
        mask_cp = singles.tile([128, NCH, 128], bf16)

        def emit_round(r):
            rnd0 = 2 * r
            nb = min(2, NCH - rnd0)
            mm = mmps.tile([128, 2, 512], f32, tag="mm", bufs=2)
            for m in range(nb):
                t = rnd0 + m
                nc.tensor.matmul(out=mm[:, m, 0:384],
                                 lhsT=basisT[:, t * 128:(t + 1) * 128],
                                 rhs=rhsST, start=True, stop=True)
            sb = mskpool.tile([128, 2, 384], bf16, tag="u")
            nc.vector.tensor_scalar(out=sb[:, :nb, :], in0=mm[:, :nb, 0:384],
                                    scalar1=0.0, scalar2=None, op0=op.is_le)
            s3 = sb[:, :, :].rearrange("p n (c three) -> p n c three", three=3)
            # last rounds' combines on DVE so the mask chain's tail doesn't
            # wait on Pool's ~1.4us/op serial latency
            eng = nc.vector if r >= 7 - dvecomb else nc.gpsimd
            pin = mskpool.tile([128, 2, 128], bf16, tag="n")
            eng.tensor_tensor(out=pin[:, :nb, :], in0=s3[:, :nb, :, 0],
                              in1=s3[:, :nb, :, 1], op=op.mult)
            q = mskpool.tile([128, 2, 128], bf16, tag="q")
            eng.tensor_tensor(out=q[:, :nb, :], in0=pin[:, :nb, :],
                              in1=s3[:, :nb, :, 2], op=op.mult)
            eng.tensor_tensor(out=pin[:, :nb, :], in0=pin[:, :nb, :],
                              in1=s3[:, :nb, :, 2], op=op.add)
            eng.tensor_tensor(out=mask_cp[:, rnd0:rnd0 + nb, :],
                              in0=pin[:, :nb, :], in1=q[:, :nb, :],
                              op=op.subtract)

        # ------------- flags: DVE bf16 chain (fast; Pool is latency-toxic) ----
        wmask = singles.tile([128, NCH, 128], bf16)
        cnt2 = singles.tile([128, NCH, 2], f32)
        wmx2 = singles.tile([128, NCH, 2], f32)
        flag2b = singles.tile([128, NCH, 2], bf16)
        oddq = singles.tile([128, NCH, 2], f32)

        def emit_cnt_parity():
            # per-scene box counts + parity (independent of wmask/wmx).
            # f32 chain: the 2^23 rne magic rounds in-register in fp32
            # (a fused bf16 +128-128 would NOT round - engines compute fp32)
            for b in range(BPC):
                nc.vector.tensor_reduce(out=cnt2[:, :, b:b + 1],
                                        in_=mask_cp[:, :, b * M:(b + 1) * M],
                                        axis=X.X, op=op.add)
            hpar = singles.tile([128, NCH, 2], f32)
            nc.vector.tensor_scalar(out=hpar, in0=cnt2, scalar1=0.5,
                                    scalar2=None, op0=op.mult)
            rpar = singles.tile([128, NCH, 2], f32)
            nc.vector.tensor_scalar(out=rpar, in0=hpar, scalar1=8388608.0,
                                    scalar2=8388608.0, op0=op.add,
                                    op1=op.subtract)
            dpar = singles.tile([128, NCH, 2], f32)
            nc.vector.tensor_tensor(out=dpar, in0=hpar, in1=rpar,
                                    op=op.subtract)
            nc.vector.tensor_tensor(out=oddq, in0=dpar, in1=dpar, op=op.mult)

        def emit_wmx_flag():
            # wmask = mask * (j+1); per-scene max = last covering box;
            # flag = 4*oddq*wmx - 1
            wb = w128b[:, :].unsqueeze(1).broadcast_to([128, NCH, 128])
            nc.vector.tensor_tensor(out=wmask, in0=mask_cp, in1=wb, op=op.mult)
            for b in range(BPC):
                nc.vector.tensor_reduce(out=wmx2[:, :, b:b + 1],
                                        in_=wmask[:, :, b * M:(b + 1) * M],
                                        axis=X.X, op=op.max)
            fl1 = singles.tile([128, NCH, 2], f32)
            nc.vector.tensor_tensor(out=fl1, in0=oddq, in1=wmx2, op=op.mult)
            nc.vector.tensor_scalar(out=flag2b, in0=fl1, scalar1=4.0,
                                    scalar2=-1.0, op0=op.mult, op1=op.add)

        def emit_onehot(b):
            fl = flag2b[:, :, b:b + 1].broadcast_to([128, NCH, 128])
            nc.vector.tensor_tensor(out=ohall[b], in0=iotaexp,
                                    in1=fl, op=op.is_equal)

        # ---------------- variance stream + segment accumulation --------------
        act_scr = singles.tile([128, D], bf16)  # discarded activation outs
        seg = segps.tile([128, 4], f32)
        tk = singles.tile([128, 1], f32)

        def emit_chunk(c):
            b, t = c // NCH, c % NCH
            csz = 128 if t < NCH - 1 else G - 128 * (NCH - 1)
            xt = xts[c]
            if c in ACT_CHUNKS:
                nc.scalar.activation(act_scr[:csz, :], xt[:csz, :], AF.Copy,
                                     accum_out=stats[:csz, c, 0:1])
                nc.scalar.activation(act_scr[:csz, :], xt[:csz, :], AF.Square,
                                     accum_out=vstg[:csz, c, 0:1])
                if c > LATE_ACT:
                    # posts deferred to after bn25 (emit_late_posts): if they
                    # sat here, a lagging ACT would stall DVE's whole bn tail
                    pass
                else:
                    # Pool f32 post (K2*sum^2); bf16 lands via the batched
                    # cast-DMA below
                    nc.gpsimd.tensor_tensor(out=tk[:csz],
                                            in0=stats[:csz, c, 0:1],
                                            in1=stats[:csz, c, 0:1],
                                            op=op.mult)
                    nc.gpsimd.tensor_tensor(out=vstg[:csz, c, 1:2],
                                            in0=tk[:csz], in1=k2c[:csz],
                                            op=op.mult)
            else:
                st = bnpool.tile([128, 4, 6], f32, tag="bnst")
                for j in range(4):
                    nc.vector.bn_stats(out=st[:csz, j:j + 1, :],
                                       in_=xt[:csz, j * 512:(j + 1) * 512])
                nc.vector.bn_aggr(out=vrhsb[:csz, c, 0:2], in_=st[:csz])

        def emit_seg(c):
            b, t = c // NCH, c % NCH
            nc.tensor.matmul(out=seg, lhsT=ohall[b][:, t, :],
                             rhs=vrhsb[:, c, 1:5],
                             start=(c == 0), stop=(c == NCHUNK - 1))

        # one is_le round per chunk slot (c=1..7) keeps the Pool flag chain
        # fed early; wmx/onehots sit late in DVE's queue (c=16..18) where
        # the flag chain is already done, so DVE never stalls mid-stream
        for c in range(NCHUNK):
            emit_chunk(c)
            if 3 <= c <= 9:
                emit_round(c - 3)
            if c == 16:
                # batched cast-DMA: early ACT chunks' (sumsq, K2*sum^2)
                # f32 staging -> bf16 rhs cols 2:4 in one SWDGE transfer
                nc.gpsimd.dma_start(out=vrhsb[:, 0:LATE_ACT:2, 2:4],
                                    in_=vstg[:, 0:LATE_ACT:2, 0:2])
            if c == 17:
                emit_cnt_parity()
            if c == 18:
                emit_wmx_flag()
            if c == 19:
                emit_onehot(0)
            if c == 20:
                emit_onehot(1)
            if c == NCHUNK - 1:
                for lc in sorted(ACT_CHUNKS):
                    if lc > LATE_ACT:
                        nc.vector.scalar_tensor_tensor(
                            out=vrhsb[:, lc, 3:4], in0=stats[:, lc, 0:1],
                            scalar=K2, in1=stats[:, lc, 0:1],
                            op0=op.mult, op1=op.mult)
                        nc.vector.tensor_copy(vrhsb[:, lc, 2:3],
                                              vstg[:, lc, 0:1])
            if c + xbufs < NCHUNK - ntail:
                emit_dma(c + xbufs)
            if c + xbufs == NCHUNK - ntail:
                # tail triggers right after the last ring trigger in the
                # Sync queue; dedicated buffers mean no WAR waits
                for ct in range(NCHUNK - ntail, NCHUNK):
                    emit_dma(ct)

        # seg matmuls after every round matmul in PE program order (a seg
        # blocked on onehots must not sit ahead of the rounds that feed them)
        for c in range(NCHUNK):
            emit_seg(c)

        # ---------------- ship raw segment sums; host finishes in f64 ---------
        segs = singles.tile([128, 4], f32)
        nc.scalar.copy(segs, seg)
        nc.scalar.dma_start(out=out_d.ap(), in_=segs)

    nc.compile()
    return nc


def _get_program(**kw):
    key = tuple(sorted(kw.items()))
    if key not in _CACHE:
        _CACHE[key] = _build_program(**kw)
    return _CACHE[key]


def _in_maps(atten_map, gt_bboxes):
    atten_map = np.ascontiguousarray(atten_map, dtype=np.float32)
    gt_bboxes = np.ascontiguousarray(gt_bboxes, dtype=np.float32)
    return [
        {
            "x": atten_map[c * BPC:(c + 1) * BPC].reshape(ROWS, D),
            "bb": gt_bboxes[c * BPC:(c + 1) * BPC].reshape(2 * M, 7),
        }
        for c in range(NCORES)
    ]


K1 = float(np.float64(D) / (D - 1))
K3 = float(np.float32(1.0 / 2048.0))


def _combine(parts):
    # parts [ncores, 128, 4]: (var_bn_sum, sumsq_sum, K2sum2_sum, count)
    p = parts.astype(np.float64)
    v = (p[:, :, 0] + K3 * p[:, :, 1] + p[:, :, 2]) * K1
    cntm = p[:, :, 3]
    valid = cntm > 0
    means = np.where(valid, v / np.maximum(cntm, 1.0), 0.0)
    total_mean = means.sum()
    total_valid = valid.sum()
    return np.array(np.float32(-total_mean / max(total_valid, 1.0)))


def _run(atten_map, gt_bboxes, trace=False, **kw):
    from concourse.bass_utils import run_bass_kernel_spmd

    nc = _get_program(**kw)
    res = run_bass_kernel_spmd(nc, _in_maps(atten_map, gt_bboxes),
                               list(range(NCORES)), trace=trace)
    parts = np.stack([res.results[c]["out"] for c in range(NCORES)])
    return _combine(parts), res


def kernel(atten_map, gt_bboxes):
    out, _ = _run(atten_map, gt_bboxes, trace=False)
    return out
